# revision 9
# baseline (speedup 1.0000x reference)
"""BlockSparseFFN (moe_routing) Trainium2 kernel — 8 NeuronCores, data-parallel over tokens.

Strategy:
- Host: compute router logits in fp64, top-16 block mask per token (matches the
  reference's f32 top-k decisions — verified the top-k sets agree with fp64 ground
  truth on this data), pass mask^T per core as an input. Pre-transpose weights.
- Device (per core, 1024 tokens): dense SwiGLU in fp32r (full PE rate, ~1.3e-4
  matmul precision): gate/up i-major psum tiles, silu*up*mask -> hidden (fp32r),
  down-projection accumulated over i-groups via gpsimd accumulate-DMA into the
  pre-zeroed token-major output. No collectives.
"""
import sys

sys.path.insert(0, "/opt/trn_rl_repo")
import numpy as np

import concourse.bass as bass
import concourse.mybir as mybir
import concourse.tile as tile
from concourse import bacc
from concourse.bass_utils import run_bass_kernel_spmd

N_CORES = 8
B, S, D = 4, 2048, 2048
N = B * S            # 8192 tokens
T = N // N_CORES     # 1024 tokens per core
I = 8192             # intermediate
NB = 64              # blocks
BS = 128             # block size
TOP_K = 16
KT = D // 128        # 16 k-tiles (contraction for gate/up)
NI = I // 128        # 64 i-tiles (= blocks)
TN = 512             # moving free dim (tokens per chunk)
NCH = T // TN        # 2 chunks
GRP = 8              # i-tiles per down group
NG = NI // GRP       # 8 groups
DC = 512             # down output d-chunk
NDC = D // DC        # 4 d-chunks

F32 = mybir.dt.float32
F32R = mybir.dt.float32r


def build_nc(repeat=1, trivial=False):
    nc = bacc.Bacc("TRN2", target_bir_lowering=False, debug=False, num_devices=N_CORES)
    xT_d = nc.dram_tensor("xT", [D, T], F32R, kind="ExternalInput")
    gT_d = nc.dram_tensor("gT", [D, I], F32R, kind="ExternalInput")
    uT_d = nc.dram_tensor("uT", [D, I], F32R, kind="ExternalInput")
    dT_d = nc.dram_tensor("dT", [I, D], F32R, kind="ExternalInput")
    mT_d = nc.dram_tensor("maskT", [NB, T], F32, kind="ExternalInput")
    out_d = nc.dram_tensor("out", [D, T], F32, kind="ExternalOutput")  # out^T (d-major)

    if trivial:
        with tile.TileContext(nc) as tc:
            with tc.tile_pool(name="tp", bufs=2) as tp:
                t = tp.tile([128, T], F32R)
                nc.sync.dma_start(t[:], xT_d.ap()[0:128, :])
                nc.sync.dma_start(out_d.ap()[0:128, 0:T], t[:].bitcast(F32))
        nc.compile()
        return nc

    with tile.TileContext(nc) as tc:
        with tc.tile_pool(name="xpool", bufs=1) as xpool, \
             tc.tile_pool(name="wpool", bufs=3) as wpool, \
             tc.tile_pool(name="hpool", bufs=1) as hpool, \
             tc.tile_pool(name="dpool", bufs=2) as dpool, \
             tc.tile_pool(name="mpool", bufs=2) as mpool, \
             tc.tile_pool(name="epool", bufs=2) as epool, \
             tc.tile_pool(name="bpool", bufs=4) as bpool, \
             tc.tile_pool(name="psgu", bufs=4, space="PSUM") as psgu, \
             tc.tile_pool(name="psdn", bufs=4, space="PSUM") as psdn:

          for _rep in range(repeat):
            # resident x^T: [128, k-tile, tokens]
            xsb = xpool.tile([128, KT, T], F32R, tag="x")
            for k in range(KT):
                nc.sync.dma_start(xsb[:, k, :], xT_d.ap()[k * 128:(k + 1) * 128, :])

            hidden = None
            for i in range(NI):
                g = i // GRP
                j = i % GRP
                if j == 0:
                    hidden = hpool.tile([128, GRP, T], F32R, tag="hid")

                # mask broadcast for block i
                stage = mpool.tile([1, T], F32, tag="stage")
                nc.sync.dma_start(stage[:], mT_d.ap()[i:i + 1, :])
                bcast = mpool.tile([128, T], F32, tag="bc")
                nc.gpsimd.partition_broadcast(bcast[:], stage[0:1, :])

                # gate/up weight tiles for i-tile: [128, KT*128] via 4 quad-k DMAs
                gw = wpool.tile([128, KT * 128], F32R, tag="gw")
                uw = wpool.tile([128, KT * 128], F32R, tag="uw")
                for q in range(4):
                    src = gT_d.ap().rearrange("(kq p) i -> kq p i", p=128)
                    nc.sync.dma_start(
                        gw[:, q * 512:(q + 1) * 512].rearrange("p (kq i) -> p kq i", i=128),
                        src[q * 4:(q + 1) * 4, :, i * 128:(i + 1) * 128].rearrange("kq p i -> p kq i"),
                    )
                    srcu = uT_d.ap().rearrange("(kq p) i -> kq p i", p=128)
                    nc.sync.dma_start(
                        uw[:, q * 512:(q + 1) * 512].rearrange("p (kq i) -> p kq i", i=128),
                        srcu[q * 4:(q + 1) * 4, :, i * 128:(i + 1) * 128].rearrange("kq p i -> p kq i"),
                    )

                # chunk-interleaved: consecutive MMs share the same stationary
                # weight tile (amortizes the fp32r self-loading weight cost)
                gpss = [psgu.tile([128, TN], F32, tag="gu", name=f"gps{i}_{ch}") for ch in range(NCH)]
                for k in range(KT):
                    for ch in range(NCH):
                        nc.tensor.matmul(gpss[ch][:], gw[:, k * 128:(k + 1) * 128],
                                         xsb[:, k, bass.ts(ch, TN)],
                                         start=(k == 0), stop=(k == KT - 1))
                upss = [psgu.tile([128, TN], F32, tag="gu", name=f"ups{i}_{ch}") for ch in range(NCH)]
                for k in range(KT):
                    for ch in range(NCH):
                        nc.tensor.matmul(upss[ch][:], uw[:, k * 128:(k + 1) * 128],
                                         xsb[:, k, bass.ts(ch, TN)],
                                         start=(k == 0), stop=(k == KT - 1))
                for ch in range(NCH):
                    tsl = bass.ts(ch, TN)
                    sg = epool.tile([128, TN], F32, tag="sg")
                    nc.scalar.activation(sg[:], gpss[ch][:], mybir.ActivationFunctionType.Silu)
                    h1 = epool.tile([128, TN], F32, tag="h1")
                    nc.vector.tensor_mul(h1[:], sg[:], upss[ch][:])
                    nc.vector.tensor_mul(hidden[:, j, tsl], h1[:], bcast[:, tsl])

                # down projection for completed group (orientation B: out^T d-major;
                # stationary = down-weight subtile, shared by the 2 chunk MMs ->
                # half the stationary weight loads vs hidden-stationary)
                if j == GRP - 1:
                    for dsub in range(D // 128):
                        dnt = dpool.tile([128, GRP, 128], F32R, tag="dw")
                        dsrc = dT_d.ap().rearrange("(it p) d -> it p d", p=128)
                        nc.sync.dma_start(
                            dnt[:],
                            dsrc[g * GRP:(g + 1) * GRP, :, dsub * 128:(dsub + 1) * 128].rearrange("it p d -> p it d"),
                        )
                        pts = [psdn.tile([128, TN], F32, tag="dn", name=f"dn{g}_{dsub}_{ch}")
                               for ch in range(NCH)]
                        for jj in range(GRP):
                            for ch in range(NCH):
                                nc.tensor.matmul(pts[ch][:], dnt[:, jj, :],
                                                 hidden[:, jj, bass.ts(ch, TN)],
                                                 start=(jj == 0), stop=(jj == GRP - 1))
                        for ch in range(NCH):
                            bounce = bpool.tile([128, TN], F32, tag="bn")
                            nc.scalar.copy(bounce[:], pts[ch][:])
                            nc.gpsimd.dma_start(
                                out_d.ap()[dsub * 128:(dsub + 1) * 128, ch * TN:(ch + 1) * TN],
                                bounce[:], accum_op=mybir.AluOpType.add)
    nc.compile()
    return nc


_CACHE = {}


def _get_nc():
    if "nc" not in _CACHE:
        _CACHE["nc"] = build_nc()
    return _CACHE["nc"]


def _host_mask(x_flat, router_w1, router_w2):
    """fp64 router + top-16; mask values replicate reference f32 arithmetic."""
    x64 = x_flat.astype(np.float64)
    r1 = x64 @ router_w1.astype(np.float64).T
    s = r1 / (1.0 + np.exp(-r1))
    lg = s @ router_w2.astype(np.float64).T          # [N, NB]
    kth = np.partition(lg, NB - TOP_K, axis=1)[:, NB - TOP_K:NB - TOP_K + 1]
    hard = (lg >= kth).astype(np.float32)
    lg32 = lg.astype(np.float32)
    p = (1.0 / (1.0 + np.exp(-lg32.astype(np.float64)))).astype(np.float32)
    return (hard - p) + p                             # f32, reference arithmetic


def kernel(x, gate_w, up_w, down_w, router_w1, router_w2):
    x = np.ascontiguousarray(np.asarray(x, dtype=np.float32))
    gate_w = np.asarray(gate_w, dtype=np.float32)
    up_w = np.asarray(up_w, dtype=np.float32)
    down_w = np.asarray(down_w, dtype=np.float32)
    router_w1 = np.asarray(router_w1, dtype=np.float32)
    router_w2 = np.asarray(router_w2, dtype=np.float32)

    x_flat = x.reshape(N, D)
    mask = _host_mask(x_flat, router_w1, router_w2)   # [N, NB] f32

    gT = np.ascontiguousarray(gate_w.T)               # [D, I]
    uT = np.ascontiguousarray(up_w.T)                 # [D, I]
    dT = np.ascontiguousarray(down_w.T)               # [I, D]

    in_maps = []
    for c in range(N_CORES):
        sl = slice(c * T, (c + 1) * T)
        in_maps.append({
            "xT": np.ascontiguousarray(x_flat[sl].T),
            "gT": gT, "uT": uT, "dT": dT,
            "maskT": np.ascontiguousarray(mask[sl].T),
        })

    nc = _get_nc()
    res = run_bass_kernel_spmd(nc, in_maps, list(range(N_CORES)))
    outT = np.concatenate([res.results[c]["out"] for c in range(N_CORES)], axis=1)
    return np.ascontiguousarray(outT.T).reshape(B, S, D)
